# revision 22
# baseline (speedup 1.0000x reference)
"""CoAttention kernel for Trainium2 (nn_CoAttention_77592879169836).

Full inputs in, full outputs out. Sharding: data-parallel over batch B=8,
one batch element per NeuronCore (8 cores), projection weights replicated.
No collectives needed.

v2 (vs baseline): host passes x1/x2 and the three projection weights
pre-transposed ([D,L] / [D,D] row-major), eliminating all 448 on-chip
PE transposes of x and W plus their PSUM round-trips. E^T tiles for the
output matmuls come from the HWDGE DMA-transpose xbar (off the PE) by
default (ETM_MODE=pe falls back to PE-mode transposes).

Per-core math (x1,x2: [L,D], L=2048, D=1024, fp32):
  q = (x1 @ Wq^T + bq) / 32 ;  k = x2 @ Wk^T + bk
  v1 = x1 @ Wv^T + bv       ;  v2 = x2 @ Wv^T + bv
  s[l,m] = q[l].k[m]                       (scale folded into Wq/bq)
  Em = exp(s + (1-mask[m])*(-1e30))        (masked exponentials, bf16)
  c2[l] = sum_m Em[l,m]                    (exp accum_out)
  c1[m] = sum_l Em[l,m]                    (ones-column matmul)
  x1_mid[m,:] = (Em^T @ v1)[m,:] * mask[m]/(c1[m]+eps)
  x2_out = (EmT @ v2) / c2[:,None]
  x1_out = (EmT @ x1_mid) / c2[:,None]
All matmuls in bf16 with fp32 PSUM accumulation.
"""
import os
import numpy as np
from contextlib import ExitStack

import concourse.bass as bass
import concourse.tile as tile
from concourse import bacc, mybir
from concourse.masks import make_identity

P = 128
B = 8
L = 2048          # L1 == L2
D = 1024
NB = 512          # matmul moving-dim size
BF = mybir.dt.bfloat16
F32 = mybir.dt.float32
ADD = mybir.AluOpType.add

NT = L // P       # 16 row tiles
ND = D // P       # 8 feature chunks
NMC = L // NB     # 4 chunks of 512
NEH = D // NB     # 2 e halves

_CACHE = {}


def _build():
    # "pe" (default): TensorE transpose-mode for E^T tiles (~275ns each).
    # "dma": xbar transpose reads from a DRAM spill of E — measured SLOWER
    # (single-queue xbar ~16GB/s stalls P5; two queues corrupt data).
    etm_mode = os.environ.get("ETM_MODE", "pe")
    scale = 1.0 / np.sqrt(np.float32(D))

    nc = bacc.Bacc("TRN2", target_bir_lowering=False, debug=False)

    x1T_in = nc.dram_tensor("x1T", [D, L], F32, kind="ExternalInput").ap()
    x2T_in = nc.dram_tensor("x2T", [D, L], F32, kind="ExternalInput").ap()
    mask_in = nc.dram_tensor("mask", [L], F32, kind="ExternalInput").ap()
    wT_in = {}
    b_in = {}
    for nm in ("q", "k", "v"):
        wT_in[nm] = nc.dram_tensor(f"W{nm}T", [D, D], F32, kind="ExternalInput").ap()
        b_in[nm] = nc.dram_tensor(f"b{nm}", [D], F32, kind="ExternalInput").ap()
    x1o_d = nc.dram_tensor("x1_out", [L, D], F32, kind="ExternalOutput").ap()
    x2o_d = nc.dram_tensor("x2_out", [L, D], F32, kind="ExternalOutput").ap()
    v1_d = nc.dram_tensor("v1_scr", [L, D], BF, kind="Internal").ap()
    E_d = nc.dram_tensor("e_scr", [L, L], BF, kind="Internal").ap()

    with tile.TileContext(nc) as tc, ExitStack() as ctx:
        const = ctx.enter_context(tc.tile_pool(name="const", bufs=1))
        vpool = ctx.enter_context(tc.tile_pool(name="vpool", bufs=1))
        qk = ctx.enter_context(tc.tile_pool(name="qk", bufs=2))

        # ---------- constants ----------
        if etm_mode == "pe":
            ident_bf = const.tile([P, P], BF)
            make_identity(nc, ident_bf[:])
        ones_row = const.tile([1, P], BF)      # K=1 lhsT of ones (replication)
        nc.any.memset(ones_row[:], 1.0)
        ones_col = const.tile([P, 1], BF)      # N=1 rhs of ones (row sums)
        nc.any.memset(ones_col[:], 1.0)

        negrow = const.tile([1, L], BF)        # (mask-1)*1e30 -> 0 or -1e30
        maskcol = const.tile([P, NT], F32)
        nc.sync.dma_start(maskcol[:], mask_in.rearrange("(t p) -> p t", p=P))

        bcol = {}
        for nm in ("q", "k"):
            raw = const.tile([P, ND], F32, tag=f"b{nm}raw", name=f"b{nm}raw")
            nc.sync.dma_start(raw[:], b_in[nm].rearrange("(c p) -> p c", p=P))
            if nm == "q":
                bcol[nm] = const.tile([P, ND], F32, tag="bqs", name="bqs")
                nc.vector.tensor_scalar_mul(bcol[nm][:], raw[:], float(scale))
            else:
                bcol[nm] = raw
        bvrow = const.tile([1, D], BF)
        c2r_sb = const.tile([P, NT], F32)      # 1/c2 per l-tile

        # ---------- persistent big tiles ----------
        v2_sb = vpool.tile([P, NT, D], BF)     # v2, [m-part, mt, e]

        # qk pool: two slots shared by qT/kT then v1r/x1mid
        qT_sb = qk.tile([P, ND, L], BF, tag="qk")
        kT_sb = qk.tile([P, ND, L], BF, tag="qk")

        # =========== P0 + P1: weight loads & projections ===========
        with ExitStack() as pctx:
            wpool = pctx.enter_context(tc.tile_pool(name="wpool", bufs=2))
            xpool = pctx.enter_context(tc.tile_pool(name="xpool", bufs=9))
            stg = pctx.enter_context(tc.tile_pool(name="stg01", bufs=2))
            ps01 = pctx.enter_context(tc.tile_pool(name="ps01", bufs=6, space="PSUM"))

            # constants that need an fp32 staging row (freed with this scope)
            mrow_f = stg.tile([1, L], F32, tag="xstg", name="mrow_f")
            nc.sync.dma_start(mrow_f[:], mask_in[None, :])
            nc.vector.tensor_scalar(
                out=negrow[:], in0=mrow_f[:], scalar1=1.0e30, scalar2=-1.0e30,
                op0=mybir.AluOpType.mult, op1=ADD)
            bvrow_f = stg.tile([1, D], F32, tag="wstg", name="bvrow_f")
            nc.sync.dma_start(bvrow_f[:], b_in["v"][None, :])
            nc.vector.tensor_copy(bvrow[:], bvrow_f[:])

            # bv replicated across partitions (K=1 matmul), bf16
            bvrep = stg.tile([P, D], BF, bufs=1)
            for eh in range(NEH):
                bvp = ps01.tile([P, NB], F32, tag="proj", name="bvp")
                nc.tensor.matmul(bvp[:], ones_row[:],
                                 bvrow[0:1, eh * NB:(eh + 1) * NB],
                                 start=True, stop=True)
                nc.vector.tensor_copy(bvrep[:, eh * NB:(eh + 1) * NB], bvp[:])

            def load_w(nm, scl, eng):
                wT = wpool.tile([P, ND, D], BF, tag="w", name=f"w{nm}T")
                for cd in range(ND):
                    wstg = stg.tile([P, D], F32, tag="wstg")
                    eng.dma_start(wstg[:], wT_in[nm][cd * P:(cd + 1) * P, :])
                    if scl is None:
                        nc.scalar.copy(wT[:, cd, :], wstg[:])
                    else:
                        nc.vector.tensor_scalar_mul(wT[:, cd, :], wstg[:], scl)
                return wT

            def load_x(xT_in, tag):
                # alternate queues so the x chunks stream in parallel with
                # the weight loads during pipeline fill
                x8 = [xpool.tile([P, L], BF, tag="x8", name=f"{tag}_{cd}")
                      for cd in range(ND)]
                for cd in range(ND):
                    eng = nc.sync if (cd % 2 == 0) else nc.scalar
                    xstg = stg.tile([P, L], F32, tag="xstg")
                    eng.dma_start(xstg[:], xT_in[cd * P:(cd + 1) * P, :])
                    nc.scalar.copy(x8[cd][:], xstg[:])
                return x8

            def x_pass(x8, wT, bc, qT_dst, v_to_dram):
                # qT / kT projection: out [e-chunk, l-block]
                for ce in range(ND):
                    for lb in range(NMC):
                        qp = ps01.tile([P, NB], F32, tag="proj", name="qp")
                        for cd in range(ND):
                            nc.tensor.matmul(
                                qp[:], wT[:, cd, ce * P:(ce + 1) * P],
                                x8[cd][:, lb * NB:(lb + 1) * NB],
                                start=(cd == 0), stop=(cd == ND - 1))
                        nc.vector.tensor_scalar_add(
                            qT_dst[:, ce, lb * NB:(lb + 1) * NB], qp[:],
                            bc[:, ce:ce + 1])
                # v projection: out [l-tile, e]
                for lt in range(NT):
                    vstg = stg.tile([P, D], BF, tag="vstg")
                    vps = [ps01.tile([P, NB], F32, tag="proj", name=f"vp{e_}")
                           for e_ in range(NEH)]
                    for cd in range(ND):
                        lhs = x8[cd][:, lt * P:(lt + 1) * P]
                        for eh in range(NEH):
                            nc.tensor.matmul(
                                vps[eh][:], lhs,
                                wvT[:, cd, eh * NB:(eh + 1) * NB],
                                start=(cd == 0), stop=(cd == ND - 1))
                    for eh in range(NEH):
                        sl = slice(eh * NB, (eh + 1) * NB)
                        dst = vstg[:, sl] if v_to_dram else v2_sb[:, lt, sl]
                        nc.vector.tensor_tensor(
                            out=dst, in0=vps[eh][:], in1=bvrep[:, sl], op=ADD)
                    if v_to_dram:
                        nc.sync.dma_start(v1_d[lt * P:(lt + 1) * P, :], vstg[:])

            # queue plan for pipeline fill: scalar serves wk first (needed by
            # the very first matmuls), sync leads with x2-even chunks; wv
            # follows the x2-evens on sync (only needed ~55us in); wq loads
            # during the x2-pass compute.
            wkT = load_w("k", None, nc.scalar)
            x8_2 = load_x(x2T_in, "x2c")
            wvT = load_w("v", None, nc.sync)
            x_pass(x8_2, wkT, bcol["k"], kT_sb, False)
            wqT = load_w("q", float(scale), nc.scalar)   # reuses wk's slot
            x8_1 = load_x(x1T_in, "x1c")
            x_pass(x8_1, wqT, bcol["q"], qT_sb, True)

        # E pool opens only after P1 staging is released (SBUF budget)
        epool = ctx.enter_context(tc.tile_pool(name="epool", bufs=1))
        E_sb = epool.tile([P, NT, L], BF)      # masked exp(s), [l-part, lt, m]

        # =========== P2: scores -> masked exp -> E, c2 ===========
        with ExitStack() as pctx:
            stg2 = pctx.enter_context(tc.tile_pool(name="stg2", bufs=2))
            ps2 = pctx.enter_context(tc.tile_pool(name="ps2", bufs=8, space="PSUM"))

            # negrow replicated across partitions, bf16 (lives only in P2)
            negrep = stg2.tile([P, L], BF, bufs=1)
            for mc in range(NMC):
                nrp = ps2.tile([P, NB], F32, tag="sp", name="nrp")
                nc.tensor.matmul(nrp[:], ones_row[:],
                                 negrow[0:1, mc * NB:(mc + 1) * NB],
                                 start=True, stop=True)
                nc.vector.tensor_copy(negrep[:, mc * NB:(mc + 1) * NB], nrp[:])

            for lt in range(NT):
                sps = [ps2.tile([P, NB], F32, tag="sp", name=f"sp{m_}")
                       for m_ in range(NMC)]
                for ce in range(ND):
                    lhs = qT_sb[:, ce, lt * P:(lt + 1) * P]
                    for mc in range(NMC):
                        nc.tensor.matmul(
                            sps[mc][:], lhs,
                            kT_sb[:, ce, mc * NB:(mc + 1) * NB],
                            start=(ce == 0), stop=(ce == ND - 1))
                c2p = stg2.tile([P, max(NMC, 2)], F32, tag="c2p")
                for mc in range(NMC):
                    sl = slice(mc * NB, (mc + 1) * NB)
                    sadd = stg2.tile([P, NB], F32, tag="sadd")
                    nc.vector.tensor_tensor(
                        out=sadd[:], in0=sps[mc][:], in1=negrep[:, sl], op=ADD)
                    nc.scalar.activation(
                        E_sb[:, lt, sl], sadd[:],
                        mybir.ActivationFunctionType.Exp,
                        accum_out=c2p[:, mc:mc + 1])
                if etm_mode != "pe":
                    # E row-block to DRAM (SWDGE) for P5's xbar transpose reads
                    nc.gpsimd.dma_start(E_d[lt * P:(lt + 1) * P, :],
                                        E_sb[:, lt, :])
                a0 = stg2.tile([P, 1], F32, tag="c2a")
                b0 = stg2.tile([P, 1], F32, tag="c2b")
                c0 = stg2.tile([P, 1], F32, tag="c2c")
                nc.vector.tensor_add(a0[:], c2p[:, 0:1], c2p[:, 1:2])
                nc.vector.tensor_add(b0[:], c2p[:, 2:3], c2p[:, 3:4])
                nc.vector.tensor_add(c0[:], a0[:], b0[:])
                nc.vector.reciprocal(c2r_sb[:, lt:lt + 1], c0[:])

        # =========== P3: x1_mid = maskcol/(c1+eps) * (Em^T @ v1) ===========
        v1r = qk.tile([P, NT, D], BF, tag="qk")
        x1mid = qk.tile([P, NT, D], BF, tag="qk")
        with ExitStack() as pctx:
            stg3 = pctx.enter_context(tc.tile_pool(name="stg3", bufs=2))
            ps3 = pctx.enter_context(tc.tile_pool(name="ps3", bufs=4, space="PSUM"))
            ps3c = pctx.enter_context(tc.tile_pool(name="ps3c", bufs=2, space="PSUM"))
            for lt in range(NT):
                # scalar queue only: the sync queue's xbar is left alone for
                # P5's (possibly overlapping) transpose reads
                nc.scalar.dma_start(v1r[:, lt, :], v1_d[lt * P:(lt + 1) * P, :])
            for mt in range(NT):
                mids = [ps3.tile([P, NB], F32, tag="mid", name=f"mid{e_}")
                        for e_ in range(NEH)]
                c1p = ps3c.tile([P, 1], F32, tag="c1")
                for lc in range(NT):
                    # mids and c1 share the same lhsT slice so the weight
                    # load is amortized over 3 consecutive matmuls
                    lhs = E_sb[:, lc, mt * P:(mt + 1) * P]
                    for eh in range(NEH):
                        nc.tensor.matmul(
                            mids[eh][:], lhs, v1r[:, lc, eh * NB:(eh + 1) * NB],
                            start=(lc == 0), stop=(lc == NT - 1))
                    nc.tensor.matmul(c1p[:], lhs, ones_col[:],
                                     start=(lc == 0), stop=(lc == NT - 1))
                c1e = stg3.tile([P, 1], F32, tag="c1e")
                c1r = stg3.tile([P, 1], F32, tag="c1r")
                r1 = stg3.tile([P, 1], F32, tag="r1")
                nc.vector.tensor_scalar_add(c1e[:], c1p[:], 1.0e-30)
                nc.vector.reciprocal(c1r[:], c1e[:])
                nc.vector.tensor_scalar_mul(r1[:], c1r[:], maskcol[:, mt:mt + 1])
                for eh in range(NEH):
                    nc.vector.tensor_scalar_mul(
                        x1mid[:, mt, eh * NB:(eh + 1) * NB], mids[eh][:], r1[:])

        # =========== P5: x2_out = EmT@v2 / c2 ; x1_out = EmT@x1mid / c2 =====
        with ExitStack() as pctx:
            stg5 = pctx.enter_context(tc.tile_pool(name="stg5", bufs=2))
            if etm_mode == "pe":
                ps5 = pctx.enter_context(
                    tc.tile_pool(name="ps5", bufs=6, space="PSUM"))
                ps5t = pctx.enter_context(
                    tc.tile_pool(name="ps5t", bufs=2, space="PSUM"))
            else:
                ps5 = pctx.enter_context(
                    tc.tile_pool(name="ps5", bufs=8, space="PSUM"))

            def etm_transpose_one(lt, etw, j):
                if etm_mode == "pe":
                    tp5 = ps5t.tile([P, P], BF, tag="tp5", name=f"tp5_{lt}_{j}")
                    nc.tensor.transpose(
                        tp5[:], E_sb[:, lt, j * P:(j + 1) * P], ident_bf[:])
                    nc.scalar.copy(etw[:, j, :], tp5[:])
                else:
                    # xbar transpose reads must be DRAM-sourced (SBUF-sourced
                    # transposes crash the exec unit); sync queue only, so no
                    # non-transpose traffic flips the xbar mode under them
                    nc.sync.dma_start(
                        etw[:, j, :],
                        E_d[lt * P:(lt + 1) * P, j * P:(j + 1) * P],
                        transpose=True)

            def new_etw(lt):
                return stg5.tile([P, NT, P], BF, tag="etw",
                                 name=f"etw{lt}", bufs=3)

            etws = {0: new_etw(0)}
            for j in range(NT):
                etm_transpose_one(0, etws[0], j)
            for lt in range(NT):
                if lt + 1 < NT:
                    etws[lt + 1] = new_etw(lt + 1)
                etw = etws.pop(lt)
                o2 = [ps5.tile([P, NB], F32, tag="o", name=f"o2_{e_}")
                      for e_ in range(NEH)]
                o1 = [ps5.tile([P, NB], F32, tag="o", name=f"o1_{e_}")
                      for e_ in range(NEH)]
                for mc in range(NT):
                    lhs = etw[:, mc, :]
                    st = (mc == 0)
                    sp_ = (mc == NT - 1)
                    for eh in range(NEH):
                        nc.tensor.matmul(
                            o2[eh][:], lhs, v2_sb[:, mc, eh * NB:(eh + 1) * NB],
                            start=st, stop=sp_)
                        nc.tensor.matmul(
                            o1[eh][:], lhs, x1mid[:, mc, eh * NB:(eh + 1) * NB],
                            start=st, stop=sp_)
                    # interleave next tile's transposes among the matmuls so
                    # the PE never runs a >3.4us transpose-only stretch (HAM
                    # would re-throttle the clock to 1.2 GHz)
                    if lt + 1 < NT:
                        etm_transpose_one(lt + 1, etws[lt + 1], mc)
                rec2 = c2r_sb[:, lt:lt + 1]
                o2stg = stg5.tile([P, D], F32, tag="o2stg")
                o1stg = stg5.tile([P, D], F32, tag="o1stg")
                for eh in range(NEH):
                    sl = slice(eh * NB, (eh + 1) * NB)
                    nc.vector.tensor_scalar_mul(o2stg[:, sl], o2[eh][:], rec2)
                    nc.vector.tensor_scalar_mul(o1stg[:, sl], o1[eh][:], rec2)
                if etm_mode == "pe":
                    # no xbar transposes in flight -> HWDGE queues are free
                    # and faster than SWDGE for the output stream
                    nc.sync.dma_start(x2o_d[lt * P:(lt + 1) * P, :], o2stg[:])
                    nc.scalar.dma_start(x1o_d[lt * P:(lt + 1) * P, :], o1stg[:])
                else:
                    # outputs via SWDGE to keep the HWDGE xbar in transpose mode
                    nc.gpsimd.dma_start(x2o_d[lt * P:(lt + 1) * P, :], o2stg[:])
                    nc.gpsimd.dma_start(x1o_d[lt * P:(lt + 1) * P, :], o1stg[:])

    nc.compile()
    return nc


def _get_nc():
    if "nc" not in _CACHE:
        _CACHE["nc"] = _build()
    return _CACHE["nc"]


def make_in_maps(x1, x2, mask, Wq, bq, Wk, bk, Wv, bv):
    x1 = np.asarray(x1, dtype=np.float32)
    x2 = np.asarray(x2, dtype=np.float32)
    mask = np.ascontiguousarray(np.asarray(mask, dtype=np.float32))
    shared = {
        "WqT": np.ascontiguousarray(np.asarray(Wq, dtype=np.float32).T),
        "bq": np.ascontiguousarray(np.asarray(bq, dtype=np.float32)),
        "WkT": np.ascontiguousarray(np.asarray(Wk, dtype=np.float32).T),
        "bk": np.ascontiguousarray(np.asarray(bk, dtype=np.float32)),
        "WvT": np.ascontiguousarray(np.asarray(Wv, dtype=np.float32).T),
        "bv": np.ascontiguousarray(np.asarray(bv, dtype=np.float32)),
    }
    return [
        {"x1T": np.ascontiguousarray(x1[c].T),
         "x2T": np.ascontiguousarray(x2[c].T),
         "mask": mask[c], **shared}
        for c in range(B)
    ]


def kernel(x1, x2, mask, Wq, bq, Wk, bk, Wv, bv):
    nc = _get_nc()
    from concourse.bass_utils import run_bass_kernel_spmd

    in_maps = make_in_maps(x1, x2, mask, Wq, bq, Wk, bk, Wv, bv)
    res = run_bass_kernel_spmd(nc, in_maps, core_ids=list(range(B)))
    x1_out = np.stack([res.results[c]["x1_out"] for c in range(B)])
    x2_out = np.stack([res.results[c]["x2_out"] for c in range(B)])
    return (x1_out, x2_out)
